# revision 27
# baseline (speedup 1.0000x reference)
"""Trainium2 Bass kernel for BroadcastingSelfAttention.

Reference computation (see problem):
    score(s,b,t) = softplus(sum_f X[s,b,f,t] * W[s,f] + bias[s])
    w(d,s,b,t)   = softmax_s(-score(s,b,t) * dist(d,s))
    out(d,b,f,t) = sum_s w(d,s,b,t) * X[s,b,f,t]

Shapes: S=64, B=16, F=64, T=96, D=1024 (= 32*32 target grid).

Sharding: B=16 split across 8 cores (2 batches per core).

Per-core dataflow, one round per t-pair (96 rounds = 2 b x 48 tp):
  * e2[(th,s)=128p, d=1024] = exp(score(s, 2*tp+th) * (-dist(d,s))) -- one
    ACT op (t-parity packs two t's into the 128 partitions; dist pre-negated
    so the ACT per-partition `scale` operand carries +score).
  * 8 matmuls, one per 128-wide d-block: stationary e2[(th,s)=128, d_blk=128
    cols], moving x2z[(th,s)=128, 130] where x2z packs both parities with
    zeros in the cross-parity quadrants and a ones column per parity (fused
    softmax denominators).  K=128 contraction -> both t outputs plus both
    denominators from ONE matmult per block (halves PE sequencer load vs
    per-parity matmuls).
  * DVE reciprocal of the 16 denominator columns.
  * drain: rank-4 TT (psum * recip broadcast) -> staging SBUF in d-partition
    layout; every BOUNCE_PERIOD-th round instead bounces psum through SBUF
    via an ACT copy + GpSimd normalize to keep ACT/DVE balanced.
  * One output DMA per 4 rounds (24 total): 8KB-contiguous runs per
    partition; host un-permutes.
Startup: X arrives in 4 t-quarters per batch so score/softplus and the x2z
build pipeline against the DMA; first round issues ~5us in.
"""

import numpy as np

import concourse.bass as bass
import concourse.tile as tile
from concourse import bacc, mybir
from concourse import bass_utils

F32 = mybir.dt.float32
BF16 = mybir.dt.bfloat16

# Problem shapes (hardcoded per contract)
S = 64          # sources
B = 16          # total batch
NCORES = 8
BL = B // NCORES  # batches per core = 2
F = 64          # features
T = 96          # time
D = 1024        # flattened target grid 32*32
DBLK = D // 128  # 8 d-blocks of 128
TP = T // 2     # 48 t-pairs
NQ = 4          # X arrives in 4 t-quarters per batch
QT = TP // NQ   # 12 t-pairs per quarter
TCH = 32        # t-chunk (stage tile holds 32 t values = 16 rounds)
NCH = T // TCH  # 3 chunks per batch
RPC = TCH // 2  # 16 rounds (t-pairs) per chunk

# MM_DT: dtype of matmul operands (e2 weights + moving x2z). bf16 halves
# weight-load time at ~0.4% relative error.
MM_DT = BF16
# OUT_DT: dtype of the staged/DMA'd output (host upcasts to f32).
OUT_DT = BF16
# Every Nth round bounces psum through SBUF via ACT-copy + GpSimd-normalize
# instead of the DVE drain (0 = never). Balances the ACT/DVE load.
BOUNCE_PERIOD = 7


def build_kernel():
    nc = bacc.Bacc("TRN2", target_bir_lowering=False, debug=False,
                   num_devices=NCORES)

    # xq[b, q, (th,s)=128, f, tq] = X[s, b, f, 2*(q*QT+tq)+th]
    x_t = nc.dram_tensor("xq", (BL, NQ, 128, F, QT), F32, kind="ExternalInput")
    # ndist_T[s, d] = -dist[d, s]  (host pre-transposed + negated)
    dist_t = nc.dram_tensor("ndist_T", (S, D), F32, kind="ExternalInput")
    w_t = nc.dram_tensor("w", (S, F), F32, kind="ExternalInput")
    bias_t = nc.dram_tensor("bias", (S, 1), F32, kind="ExternalInput")
    # Output in hardware-native layout (host un-permutes):
    # [b, ch, tlh, p, dblk, f, tlo] -> out[dblk*128+p, b, f, ch*32+tlh*8+tlo]
    out_t = nc.dram_tensor("out_hw", (BL, NCH, 4, 128, DBLK, F, 8),
                           OUT_DT, kind="ExternalOutput")

    def dram_ap(t, offset, ap):
        base = t.ap()
        return bass.AP(tensor=base.tensor, offset=offset, ap=ap)

    with tile.TileContext(nc) as tc:
        with (
            tc.tile_pool(name="statics", bufs=1) as statics,
            tc.tile_pool(name="ztp", bufs=2) as ztp,
            tc.tile_pool(name="e2p", bufs=10) as e2p,
            tc.tile_pool(name="stage", bufs=2) as stagep,
            tc.tile_pool(name="small", bufs=4) as small,
            tc.tile_pool(name="tmpp", bufs=4) as tmpp,
            tc.tile_pool(name="psum", bufs=3, space="PSUM") as psump,
            tc.tile_pool(name="denp", bufs=1, space="PSUM") as denp,
        ):
            # ---- static tiles -------------------------------------------
            # xsc[b][p=(th,s), q, f, tq]: f32 X for the score path
            xsc = [statics.tile([128, NQ, F, QT], F32, name=f"xsc{i}")
                   for i in range(BL)]
            # x2z[b][p=(th,s), tp, 128]: bf16 moving operand; cols z*64+f,
            # z==th rows hold X, z!=th rows are zero.
            x2z = [statics.tile([128, TP, 128], MM_DT, name=f"x2z{i}")
                   for i in range(BL)]
            # ones2[(th,s), z]: 1 where z==th -- moving operand of the tiny
            # denominator matmuls (den[d, t] = sum_s e2[(t,s), d])
            ones2 = statics.tile([128, 2], MM_DT)
            # ndist2[(th,s), d] = -dist(d,s), replicated across t-parity
            ndist2 = statics.tile([128, D], F32)
            w2 = statics.tile([128, F], F32)
            bias2 = statics.tile([128, 1], F32)
            sc = [statics.tile([128, TP], F32, name=f"sc{i}") for i in range(BL)]

            # ---- input DMAs (sync queue, in priority order) --------------
            def xq_dma(b, q):
                eng = nc.sync
                eng.dma_start(
                    out=xsc[b][:, q, :, :],
                    in_=dram_ap(x_t, (b * NQ + q) * 128 * F * QT,
                                [[F * QT, 128], [1, F * QT]]),
                )

            xq_dma(0, 0)
            for th in range(2):
                nc.sync.dma_start(
                    out=w2[th * S: (th + 1) * S, :],
                    in_=dram_ap(w_t, 0, [[F, S], [1, F]]),
                )
                nc.sync.dma_start(
                    out=bias2[th * S: (th + 1) * S, :],
                    in_=dram_ap(bias_t, 0, [[1, S], [0, 1]]),
                )
            for th in range(2):
                nc.sync.dma_start(
                    out=ndist2[th * S: (th + 1) * S, :],
                    in_=dram_ap(dist_t, 0, [[D, S], [1, D]]),
                )
            for q in range(1, NQ):
                xq_dma(0, q)
            for q in range(NQ):
                xq_dma(1, q)

            # softplus exp accumulators (Ln batched to avoid ACT
            # exp<->ln table thrash against the per-round e2 Exps)
            ez = [statics.tile([128, TP], F32, name=f"ez{i}")
                  for i in range(BL)]

            # ---- per-quarter score + x2z build --------------------------
            # x2z copies always on Pool; z/add on `veng`; reduce DVE-only.
            def build_quarter(b, q, veng):
                q0 = q * QT
                nc.gpsimd.tensor_copy(
                    x2z[b][0:S, q0:q0 + QT, 0:F],
                    xsc[b][0:S, q, :, :].rearrange("p f t -> p t f"),
                )
                nc.gpsimd.tensor_copy(
                    x2z[b][S:128, q0:q0 + QT, F:128],
                    xsc[b][S:128, q, :, :].rearrange("p f t -> p t f"),
                )
                # score: z = sum_f x*w
                zt = ztp.tile([128, QT, F], F32, tag=f"zt{b}")
                veng.tensor_tensor(
                    out=zt[:],
                    in0=xsc[b][:, q, :, :].rearrange("p f t -> p t f"),
                    in1=w2[:].unsqueeze(1).broadcast_to([128, QT, F]),
                    op=mybir.AluOpType.mult,
                )
                z = ztp.tile([128, QT], F32, tag=f"z{b}")
                # free-axis reduce is DVE-only (GpSimd can't)
                nc.vector.reduce_sum(out=z[:], in_=zt[:],
                                     axis=mybir.AxisListType.X)
                # ez = 1 + exp(z+bias)   (softplus part 1)
                nc.scalar.activation(
                    out=ez[b][:, q0:q0 + QT], in_=z[:],
                    func=mybir.ActivationFunctionType.Exp,
                    bias=bias2[:, 0:1], scale=1.0,
                )
                veng.tensor_scalar_add(
                    ez[b][:, q0:q0 + QT], ez[b][:, q0:q0 + QT], 1.0)

            def score_ln(b, q0, q1):
                # softplus part 2: sc = ln(ez), batched over quarters
                nc.scalar.activation(
                    out=sc[b][:, q0 * QT:q1 * QT], in_=ez[b][:, q0 * QT:q1 * QT],
                    func=mybir.ActivationFunctionType.Ln,
                )

            # b=0 zero quadrants on idle DVE at t=0 (big memsets, but DVE
            # has nothing else until the first xq quarter lands)
            nc.vector.memset(x2z[0][S:128, :, 0:F], 0.0)
            nc.vector.memset(x2z[0][0:S, :, F:128], 0.0)
            nc.vector.memset(ones2[0:S, 0:1], 1.0)
            nc.vector.memset(ones2[0:S, 1:2], 0.0)
            nc.vector.memset(ones2[S:128, 0:1], 0.0)
            nc.vector.memset(ones2[S:128, 1:2], 1.0)
            # b=1 zero quadrant A rides Pool's idle t=0 window
            nc.gpsimd.memset(x2z[1][S:128, :, 0:F], 0.0)
            build_quarter(0, 0, nc.vector)
            score_ln(0, 0, 1)
            # rest of b=1 x2z init between the b=0 quarter builds (Pool is
            # gated on the xq DMAs anyway)
            nc.gpsimd.memset(x2z[1][0:S, :, F:128], 0.0)
            for q in range(1, NQ):
                build_quarter(0, q, nc.vector)
            score_ln(0, 1, NQ)

            # ---- rounds --------------------------------------------------
            # Drains are deferred one round: round r emits recip(r) then
            # drain(r-1), so drain never waits on the recip->TT semaphore
            # (rc(r-1)'s update fired long ago).  Output DMAs shift one
            # round later to stay behind their last drain.
            # den[p, slot, dblk, t]: static 1-bank psum tile, slots
            # alternate by round parity (WAR tracked per-AP)
            den = denp.tile([128, 2, DBLK, 2], F32)
            stage = None
            pend_bounce = None  # (num_ap, rc_b, out_ap) for ACT+Pool path
            pend_drain = None   # (num_ap, rc_b, out_ap) for DVE path
            hp = DBLK * F * 8   # 4096 out elements per partition per tlh

            def emit_dma(st, b, ch, tlh):
                nc.sync.dma_start(
                    out=dram_ap(
                        out_t,
                        ((b * NCH + ch) * 4 + tlh) * 128 * hp,
                        [[hp, 128], [1, hp]],
                    ),
                    in_=st[:, tlh, :, :, :],
                )

            def emit_bounce(rec):
                p_num, p_rcb, p_out = rec
                tmp = tmpp.tile([128, DBLK, 2, F], F32, tag="bnc")
                nc.scalar.activation(
                    out=tmp[:], in_=p_num,
                    func=mybir.ActivationFunctionType.Copy,
                )
                nc.gpsimd.tensor_tensor(
                    out=p_out, in0=tmp[:], in1=p_rcb,
                    op=mybir.AluOpType.mult,
                )

            pend_drains = []  # (num_ap, out_ap, slot) awaiting pair recipB
            rcB = None

            for r in range(BL * TP):
                b, tp = divmod(r, TP)
                ch, rr = divmod(tp, RPC)
                tlh, r4 = divmod(rr, 4)
                tlo = 2 * r4

                if rr == 0:
                    stage_prev, stage = stage, stagep.tile(
                        [128, 4, DBLK, F, 8], OUT_DT)

                # e2[(th,s), d] = exp(score * -dist)
                e2 = e2p.tile([128, D], MM_DT)
                nc.scalar.activation(
                    out=e2[:], in_=ndist2[:],
                    func=mybir.ActivationFunctionType.Exp,
                    scale=sc[b][:, tp: tp + 1],
                )

                # pm[d%128=128p, dblk, 128]: cols t*64+f, exactly 2 PSUM
                # banks -> 3 pool bufs, so matmuls run a full round ahead.
                pm = psump.tile([128, DBLK, 128], F32, tag="pm")
                for dblk in range(DBLK):
                    nc.tensor.matmul(
                        out=pm[:, dblk, :],
                        lhsT=e2[:, dblk * 128: (dblk + 1) * 128],
                        rhs=x2z[b][:, tp, :],
                        start=True, stop=True,
                    )
                    # denominator: same stationary, ones moving
                    nc.tensor.matmul(
                        out=den[:, r % 2, dblk, :],
                        lhsT=e2[:, dblk * 128: (dblk + 1) * 128],
                        rhs=ones2[:],
                        start=True, stop=True,
                    )

                # one reciprocal per round PAIR (both den slots at once):
                # only one recip->drain semaphore wait per two drains
                if r % 2 == 1:
                    rcB = small.tile([128, 2, DBLK, 2], F32, tag="rc")
                    nc.vector.reciprocal(out=rcB[:], in_=den[:])

                # deferred bounce (ACT copy + Pool normalize): its pair's
                # rcB is emitted by now
                if pend_bounce is not None:
                    emit_bounce(pend_bounce)
                    pend_bounce = None

                # this round's drain record: stage[tlh, dblk, f, tlo+t]
                num_ap = pm[:].rearrange("p d (t x) -> p d t x", t=2)
                out_ap = stage[:, tlh, :, :, tlo: tlo + 2].rearrange(
                    "p d x t -> p d t x")
                pend_drains.append((num_ap, out_ap, r % 2,
                                    BOUNCE_PERIOD
                                    and r % BOUNCE_PERIOD == BOUNCE_PERIOD - 1))
                if r % 2 == 1:
                    for p_num, p_out, slot, bounce in pend_drains:
                        p_rcb = rcB[:, slot].unsqueeze(3).broadcast_to(
                            [128, DBLK, 2, F])
                        if bounce:
                            pend_bounce = (p_num, p_rcb, p_out)
                        else:
                            nc.vector.tensor_tensor(
                                out=p_out, in0=p_num, in1=p_rcb,
                                op=mybir.AluOpType.mult,
                            )
                    pend_drains = []

                # ---- output DMA, one round after its tlh's last drain ---
                # (flush any pending bounce first: the DMA may read the
                # stage slot the bounce writes)
                if rr in (4, 8, 12):
                    if pend_bounce is not None:
                        emit_bounce(pend_bounce)
                        pend_bounce = None
                    emit_dma(stage, b, ch, tlh - 1)
                elif rr == 0 and r > 0:
                    if pend_bounce is not None:
                        emit_bounce(pend_bounce)
                        pend_bounce = None
                    pb, pch = divmod((r - 1) // RPC, NCH)
                    emit_dma(stage_prev, pb, pch, 3)

                # b=1 prep on Pool, spread over early-round gaps (kept
                # clear of bounce rounds so bounces drain promptly)
                if r in (11, 16, 21, 26):
                    q = (11, 16, 21, 26).index(r)
                    build_quarter(1, q, nc.gpsimd)
                elif r == 28:
                    score_ln(1, 0, NQ)

            # tail: flush the last bounce and the final chunk DMA
            if pend_bounce is not None:
                emit_bounce(pend_bounce)
            emit_dma(stage, BL - 1, NCH - 1, 3)

    nc.compile()
    return nc


_NC_CACHE = None


def _get_nc():
    global _NC_CACHE
    if _NC_CACHE is None:
        _NC_CACHE = build_kernel()
    return _NC_CACHE


def make_inputs(X, dist, attention_weight, attention_bias):
    """Host-side marshaling: full inputs -> per-core input maps."""
    X = np.asarray(X, dtype=np.float32)                                # (S,B,F,T)
    dist_np = np.asarray(dist, dtype=np.float32).reshape(-1, S)        # (D,S)
    ndist_T = np.ascontiguousarray(-dist_np.T)                         # (S,D)
    w_np = np.ascontiguousarray(np.asarray(attention_weight, np.float32))
    bias_np = np.ascontiguousarray(
        np.asarray(attention_bias, np.float32).reshape(S, 1))
    # xq[b, q, (th,s), f, tq] = X[s, b, f, 2*(q*QT+tq)+th]
    # X -> (S, B, F, NQ, QT, 2) -> (B, NQ, 2, S, F, QT)
    xq_full = np.ascontiguousarray(
        X.reshape(S, B, F, NQ, QT, 2).transpose(1, 3, 5, 0, 2, 4))
    in_maps = []
    for c in range(NCORES):
        in_maps.append({
            "xq": np.ascontiguousarray(
                xq_full[c * BL: (c + 1) * BL]).reshape(BL, NQ, 128, F, QT),
            "ndist_T": ndist_T,
            "w": w_np,
            "bias": bias_np,
        })
    return in_maps


def unpack_out(hw):
    """out_hw (BL,NCH,4,128,DBLK,F,8) -> (D, BL, F, T) f32."""
    # [b, ch, tlh, p, dblk, f, tlo] -> [dblk, p, b, f, ch, tlh, tlo]
    return (np.asarray(hw).astype(np.float32)
            .transpose(4, 3, 0, 5, 1, 2, 6)
            .reshape(D, BL, F, T))


def kernel(X, dist, attention_weight, attention_bias):
    in_maps = make_inputs(X, dist, attention_weight, attention_bias)
    nc = _get_nc()
    res = bass_utils.run_bass_kernel_spmd(nc, in_maps, core_ids=list(range(NCORES)))
    out = np.empty((D, B, F, T), dtype=np.float32)
    for c in range(NCORES):
        out[:, c * BL: (c + 1) * BL] = unpack_out(res.results[c]["out_hw"])
    return out.reshape(32, 32, B, F, T)


# revision 29
# speedup vs baseline: 1.0829x; 1.0829x over previous
"""Trainium2 Bass kernel for BroadcastingSelfAttention.

Reference computation (see problem):
    score(s,b,t) = softplus(sum_f X[s,b,f,t] * W[s,f] + bias[s])
    w(d,s,b,t)   = softmax_s(-score(s,b,t) * dist(d,s))
    out(d,b,f,t) = sum_s w(d,s,b,t) * X[s,b,f,t]

Shapes: S=64, B=16, F=64, T=96, D=1024 (= 32*32 target grid).

Sharding: B=16 split across 8 cores (2 batches per core).

Per-core dataflow, one round per t-pair (96 rounds = 2 b x 48 tp):
  * e2[(th,s)=128p, d=1024] = exp(score(s, 2*tp+th) * (-dist(d,s))) -- one
    ACT op (t-parity packs two t's into the 128 partitions; dist pre-negated
    so the ACT per-partition `scale` operand carries +score).
  * 8 matmuls, one per 128-wide d-block: stationary e2[(th,s)=128, d_blk=128
    cols], moving x2z[(th,s)=128, 130] where x2z packs both parities with
    zeros in the cross-parity quadrants and a ones column per parity (fused
    softmax denominators).  K=128 contraction -> both t outputs plus both
    denominators from ONE matmult per block (halves PE sequencer load vs
    per-parity matmuls).
  * DVE reciprocal of the 16 denominator columns.
  * drain: rank-4 TT (psum * recip broadcast) -> staging SBUF in d-partition
    layout; every BOUNCE_PERIOD-th round instead bounces psum through SBUF
    via an ACT copy + GpSimd normalize to keep ACT/DVE balanced.
  * One output DMA per 4 rounds (24 total): 8KB-contiguous runs per
    partition; host un-permutes.
Startup: X arrives in 4 t-quarters per batch so score/softplus and the x2z
build pipeline against the DMA; first round issues ~5us in.
"""

import numpy as np

import concourse.bass as bass
import concourse.tile as tile
from concourse import bacc, mybir
from concourse import bass_utils

F32 = mybir.dt.float32
BF16 = mybir.dt.bfloat16

# Problem shapes (hardcoded per contract)
S = 64          # sources
B = 16          # total batch
NCORES = 8
BL = B // NCORES  # batches per core = 2
F = 64          # features
T = 96          # time
D = 1024        # flattened target grid 32*32
DBLK = D // 128  # 8 d-blocks of 128
TP = T // 2     # 48 t-pairs
NQ = 4          # X arrives in 4 t-quarters per batch
QT = TP // NQ   # 12 t-pairs per quarter
TCH = 32        # t-chunk (stage tile holds 32 t values = 16 rounds)
NCH = T // TCH  # 3 chunks per batch
RPC = TCH // 2  # 16 rounds (t-pairs) per chunk

# MM_DT: dtype of matmul operands (e2 weights + moving x2z). bf16 halves
# weight-load time at ~0.4% relative error.
MM_DT = BF16
# OUT_DT: dtype of the staged/DMA'd output (host upcasts to f32).
OUT_DT = BF16
# Every Nth round bounces psum through SBUF via ACT-copy + GpSimd-normalize
# instead of the DVE drain (0 = never). Balances the ACT/DVE load.
BOUNCE_PERIOD = 7


def build_kernel():
    nc = bacc.Bacc("TRN2", target_bir_lowering=False, debug=False,
                   num_devices=NCORES)

    # xq[b, q, (th,s)=128, f, tq] = X[s, b, f, 2*(q*QT+tq)+th]
    x_t = nc.dram_tensor("xq", (BL, NQ, 128, F, QT), F32, kind="ExternalInput")
    # ndist_T[s, d] = -dist[d, s]  (host pre-transposed + negated)
    dist_t = nc.dram_tensor("ndist_T", (S, D), F32, kind="ExternalInput")
    w_t = nc.dram_tensor("w", (S, F), F32, kind="ExternalInput")
    bias_t = nc.dram_tensor("bias", (S, 1), F32, kind="ExternalInput")
    # Output in hardware-native layout (host un-permutes):
    # [b, ch, tlh, p, dblk, f, tlo] -> out[dblk*128+p, b, f, ch*32+tlh*8+tlo]
    out_t = nc.dram_tensor("out_hw", (BL, NCH, 4, 128, DBLK, F, 8),
                           OUT_DT, kind="ExternalOutput")

    def dram_ap(t, offset, ap):
        base = t.ap()
        return bass.AP(tensor=base.tensor, offset=offset, ap=ap)

    with tile.TileContext(nc) as tc:
        with (
            tc.tile_pool(name="statics", bufs=1) as statics,
            tc.tile_pool(name="ztp", bufs=2) as ztp,
            tc.tile_pool(name="e2p", bufs=6) as e2p,
            tc.tile_pool(name="stage", bufs=2) as stagep,
            tc.tile_pool(name="small", bufs=4) as small,
            tc.tile_pool(name="tmpp", bufs=4) as tmpp,
            tc.tile_pool(name="psum", bufs=3, space="PSUM") as psump,
            tc.tile_pool(name="denp", bufs=1, space="PSUM") as denp,
        ):
            # ---- static tiles -------------------------------------------
            # xsc[b][p=(th,s), q, f, tq]: f32 X for the score path
            xsc = [statics.tile([128, NQ, F, QT], F32, name=f"xsc{i}")
                   for i in range(BL)]
            # x2z[b][p=(th,s), tp, 128]: bf16 moving operand; cols z*64+f,
            # z==th rows hold X, z!=th rows are zero.
            x2z = [statics.tile([128, TP, 128], MM_DT, name=f"x2z{i}")
                   for i in range(BL)]
            # ones2[(th,s), z]: 1 where z==th -- moving operand of the tiny
            # denominator matmuls (den[d, t] = sum_s e2[(t,s), d])
            ones2 = statics.tile([128, 2], MM_DT)
            # ndist2[(th,s), d] = -dist(d,s), replicated across t-parity
            ndist2 = statics.tile([128, D], F32)
            w2 = statics.tile([128, F], F32)
            bias2 = statics.tile([128, 1], F32)
            sc = [statics.tile([128, TP], F32, name=f"sc{i}") for i in range(BL)]

            # ---- input DMAs (sync queue, in priority order) --------------
            def xq_dma(b, q):
                eng = nc.sync
                eng.dma_start(
                    out=xsc[b][:, q, :, :],
                    in_=dram_ap(x_t, (b * NQ + q) * 128 * F * QT,
                                [[F * QT, 128], [1, F * QT]]),
                )

            xq_dma(0, 0)
            for th in range(2):
                nc.sync.dma_start(
                    out=w2[th * S: (th + 1) * S, :],
                    in_=dram_ap(w_t, 0, [[F, S], [1, F]]),
                )
                nc.sync.dma_start(
                    out=bias2[th * S: (th + 1) * S, :],
                    in_=dram_ap(bias_t, 0, [[1, S], [0, 1]]),
                )
            for th in range(2):
                nc.sync.dma_start(
                    out=ndist2[th * S: (th + 1) * S, :],
                    in_=dram_ap(dist_t, 0, [[D, S], [1, D]]),
                )
            for q in range(1, NQ):
                xq_dma(0, q)
            for q in range(NQ):
                xq_dma(1, q)

            # ---- per-quarter score + x2z build --------------------------
            # x2z copies always on Pool; z/add on `veng`; reduce DVE-only.
            def build_quarter(b, q, veng):
                q0 = q * QT
                nc.gpsimd.tensor_copy(
                    x2z[b][0:S, q0:q0 + QT, 0:F],
                    xsc[b][0:S, q, :, :].rearrange("p f t -> p t f"),
                )
                nc.gpsimd.tensor_copy(
                    x2z[b][S:128, q0:q0 + QT, F:128],
                    xsc[b][S:128, q, :, :].rearrange("p f t -> p t f"),
                )
                # score: z = sum_f x*w
                zt = ztp.tile([128, QT, F], F32, tag=f"zt{b}")
                veng.tensor_tensor(
                    out=zt[:],
                    in0=xsc[b][:, q, :, :].rearrange("p f t -> p t f"),
                    in1=w2[:].unsqueeze(1).broadcast_to([128, QT, F]),
                    op=mybir.AluOpType.mult,
                )
                z = ztp.tile([128, QT], F32, tag=f"z{b}")
                # free-axis reduce is DVE-only (GpSimd can't)
                nc.vector.reduce_sum(out=z[:], in_=zt[:],
                                     axis=mybir.AxisListType.X)
                # sc = softplus(z + bias) in one table op
                nc.scalar.activation(
                    out=sc[b][:, q0:q0 + QT], in_=z[:],
                    func=mybir.ActivationFunctionType.Softplus,
                    bias=bias2[:, 0:1], scale=1.0,
                )

            # b=0 zero quadrants on idle DVE at t=0 (big memsets, but DVE
            # has nothing else until the first xq quarter lands)
            nc.vector.memset(x2z[0][S:128, :, 0:F], 0.0)
            nc.vector.memset(x2z[0][0:S, :, F:128], 0.0)
            nc.vector.memset(ones2[0:S, 0:1], 1.0)
            nc.vector.memset(ones2[0:S, 1:2], 0.0)
            nc.vector.memset(ones2[S:128, 0:1], 0.0)
            nc.vector.memset(ones2[S:128, 1:2], 1.0)
            # b=1 zero quadrant A rides Pool's idle t=0 window
            nc.gpsimd.memset(x2z[1][S:128, :, 0:F], 0.0)
            build_quarter(0, 0, nc.vector)
            # rest of b=1 x2z init between the b=0 quarter builds (Pool is
            # gated on the xq DMAs anyway)
            nc.gpsimd.memset(x2z[1][0:S, :, F:128], 0.0)
            for q in range(1, NQ):
                build_quarter(0, q, nc.vector)

            # ---- rounds --------------------------------------------------
            # Drains are deferred one round: round r emits recip(r) then
            # drain(r-1), so drain never waits on the recip->TT semaphore
            # (rc(r-1)'s update fired long ago).  Output DMAs shift one
            # round later to stay behind their last drain.
            # den[p, slot, dblk, t]: static 1-bank psum tile, slots
            # alternate by round parity (WAR tracked per-AP)
            den = denp.tile([128, 2, DBLK, 2], F32)
            stage = None
            pend_bounce = None  # (num_ap, rc_b, out_ap) for ACT+Pool path
            pend_drain = None   # (num_ap, rc_b, out_ap) for DVE path
            hp = DBLK * F * 8   # 4096 out elements per partition per tlh

            def emit_dma(st, b, ch, tlh):
                nc.sync.dma_start(
                    out=dram_ap(
                        out_t,
                        ((b * NCH + ch) * 4 + tlh) * 128 * hp,
                        [[hp, 128], [1, hp]],
                    ),
                    in_=st[:, tlh, :, :, :],
                )

            def emit_bounce(rec):
                p_num, p_rcb, p_out = rec
                tmp = tmpp.tile([128, DBLK, 2, F], F32, tag="bnc")
                nc.scalar.activation(
                    out=tmp[:], in_=p_num,
                    func=mybir.ActivationFunctionType.Copy,
                )
                nc.gpsimd.tensor_tensor(
                    out=p_out, in0=tmp[:], in1=p_rcb,
                    op=mybir.AluOpType.mult,
                )

            pend_drains = []  # (num_ap, out_ap, slot) awaiting pair recipB
            rcB = None

            for r in range(BL * TP):
                b, tp = divmod(r, TP)
                ch, rr = divmod(tp, RPC)
                tlh, r4 = divmod(rr, 4)
                tlo = 2 * r4

                if rr == 0:
                    stage_prev, stage = stage, stagep.tile(
                        [128, 4, DBLK, F, 8], OUT_DT)

                # e2[(th,s), d] = exp(score * -dist)
                e2 = e2p.tile([128, D], MM_DT)
                nc.scalar.activation(
                    out=e2[:], in_=ndist2[:],
                    func=mybir.ActivationFunctionType.Exp,
                    scale=sc[b][:, tp: tp + 1],
                )

                # pm[d%128=128p, dblk, 128]: cols t*64+f, exactly 2 PSUM
                # banks -> 3 pool bufs, so matmuls run a full round ahead.
                pm = psump.tile([128, DBLK, 128], F32, tag="pm")
                for dblk in range(DBLK):
                    nc.tensor.matmul(
                        out=pm[:, dblk, :],
                        lhsT=e2[:, dblk * 128: (dblk + 1) * 128],
                        rhs=x2z[b][:, tp, :],
                        start=True, stop=True,
                    )
                    # denominator: same stationary, ones moving
                    nc.tensor.matmul(
                        out=den[:, r % 2, dblk, :],
                        lhsT=e2[:, dblk * 128: (dblk + 1) * 128],
                        rhs=ones2[:],
                        start=True, stop=True,
                    )

                # one reciprocal per round PAIR (both den slots at once):
                # only one recip->drain semaphore wait per two drains
                if r % 2 == 1:
                    rcB = small.tile([128, 2, DBLK, 2], F32, tag="rc")
                    nc.vector.reciprocal(out=rcB[:], in_=den[:])

                # deferred bounce (ACT copy + Pool normalize): its pair's
                # rcB is emitted by now
                if pend_bounce is not None:
                    emit_bounce(pend_bounce)
                    pend_bounce = None

                # this round's drain record: stage[tlh, dblk, f, tlo+t]
                num_ap = pm[:].rearrange("p d (t x) -> p d t x", t=2)
                out_ap = stage[:, tlh, :, :, tlo: tlo + 2].rearrange(
                    "p d x t -> p d t x")
                pend_drains.append((num_ap, out_ap, r % 2,
                                    BOUNCE_PERIOD
                                    and r % BOUNCE_PERIOD == BOUNCE_PERIOD - 1))
                if r % 2 == 1:
                    for p_num, p_out, slot, bounce in pend_drains:
                        p_rcb = rcB[:, slot].unsqueeze(3).broadcast_to(
                            [128, DBLK, 2, F])
                        if bounce:
                            pend_bounce = (p_num, p_rcb, p_out)
                        else:
                            nc.vector.tensor_tensor(
                                out=p_out, in0=p_num, in1=p_rcb,
                                op=mybir.AluOpType.mult,
                            )
                    pend_drains = []

                # ---- output DMA, one round after its tlh's last drain ---
                # (flush any pending bounce first: the DMA may read the
                # stage slot the bounce writes)
                if rr in (4, 8, 12):
                    if pend_bounce is not None:
                        emit_bounce(pend_bounce)
                        pend_bounce = None
                    emit_dma(stage, b, ch, tlh - 1)
                elif rr == 0 and r > 0:
                    if pend_bounce is not None:
                        emit_bounce(pend_bounce)
                        pend_bounce = None
                    pb, pch = divmod((r - 1) // RPC, NCH)
                    emit_dma(stage_prev, pb, pch, 3)

                # b=1 prep on Pool, spread over early-round gaps (kept
                # clear of bounce rounds so bounces drain promptly)
                if r in (11, 16, 21, 26):
                    q = (11, 16, 21, 26).index(r)
                    build_quarter(1, q, nc.gpsimd)

            # tail: flush the last bounce and the final chunk DMA
            if pend_bounce is not None:
                emit_bounce(pend_bounce)
            emit_dma(stage, BL - 1, NCH - 1, 3)

    nc.compile()
    return nc


_NC_CACHE = None


def _get_nc():
    global _NC_CACHE
    if _NC_CACHE is None:
        _NC_CACHE = build_kernel()
    return _NC_CACHE


def make_inputs(X, dist, attention_weight, attention_bias):
    """Host-side marshaling: full inputs -> per-core input maps."""
    X = np.asarray(X, dtype=np.float32)                                # (S,B,F,T)
    dist_np = np.asarray(dist, dtype=np.float32).reshape(-1, S)        # (D,S)
    ndist_T = np.ascontiguousarray(-dist_np.T)                         # (S,D)
    w_np = np.ascontiguousarray(np.asarray(attention_weight, np.float32))
    bias_np = np.ascontiguousarray(
        np.asarray(attention_bias, np.float32).reshape(S, 1))
    # xq[b, q, (th,s), f, tq] = X[s, b, f, 2*(q*QT+tq)+th]
    # X -> (S, B, F, NQ, QT, 2) -> (B, NQ, 2, S, F, QT)
    xq_full = np.ascontiguousarray(
        X.reshape(S, B, F, NQ, QT, 2).transpose(1, 3, 5, 0, 2, 4))
    in_maps = []
    for c in range(NCORES):
        in_maps.append({
            "xq": np.ascontiguousarray(
                xq_full[c * BL: (c + 1) * BL]).reshape(BL, NQ, 128, F, QT),
            "ndist_T": ndist_T,
            "w": w_np,
            "bias": bias_np,
        })
    return in_maps


def unpack_out(hw):
    """out_hw (BL,NCH,4,128,DBLK,F,8) -> (D, BL, F, T) f32."""
    # [b, ch, tlh, p, dblk, f, tlo] -> [dblk, p, b, f, ch, tlh, tlo]
    return (np.asarray(hw).astype(np.float32)
            .transpose(4, 3, 0, 5, 1, 2, 6)
            .reshape(D, BL, F, T))


def kernel(X, dist, attention_weight, attention_bias):
    in_maps = make_inputs(X, dist, attention_weight, attention_bias)
    nc = _get_nc()
    res = bass_utils.run_bass_kernel_spmd(nc, in_maps, core_ids=list(range(NCORES)))
    out = np.empty((D, B, F, T), dtype=np.float32)
    for c in range(NCORES):
        out[:, c * BL: (c + 1) * BL] = unpack_out(res.results[c]["out_hw"])
    return out.reshape(32, 32, B, F, T)


# revision 35
# speedup vs baseline: 1.0863x; 1.0031x over previous
"""Trainium2 Bass kernel for BroadcastingSelfAttention.

Reference computation (see problem):
    score(s,b,t) = softplus(sum_f X[s,b,f,t] * W[s,f] + bias[s])
    w(d,s,b,t)   = softmax_s(-score(s,b,t) * dist(d,s))
    out(d,b,f,t) = sum_s w(d,s,b,t) * X[s,b,f,t]

Shapes: S=64, B=16, F=64, T=96, D=1024 (= 32*32 target grid).

Sharding: B=16 split across 8 cores (2 batches per core).

Per-core dataflow, one round per t-pair (96 rounds = 2 b x 48 tp):
  * e2[(th,s)=128p, d=1024] = exp(score(s, 2*tp+th) * (-dist(d,s))) -- one
    ACT op (t-parity packs two t's into the 128 partitions; dist pre-negated
    so the ACT per-partition `scale` operand carries +score).
  * Per 128-wide d-block: one K=128 matmul with stationary e2 and moving
    x2z[(th,s)=128, 128] (both parities packed, zeros in the cross-parity
    quadrants -> both t outputs from one matmult; halves PE sequencer load
    vs per-parity matmuls), plus a tiny 2-column denominator matmult with a
    parity-masked ones operand into a rotating 1-bank psum slot.
  * pm is exactly 2 psum banks -> 3 pool bufs: matmuls run a full round
    ahead of the drains.
  * ONE DVE reciprocal per round PAIR (reads both den slots), so only one
    recip->drain semaphore wait per two drains; drains are emitted at odd
    rounds for the (even, odd) pair.
  * drain: rank-4 TT (psum * recip broadcast) -> staging SBUF in
    d-partition layout; every BOUNCE_PERIOD-th round instead bounces psum
    through SBUF via an ACT copy + GpSimd normalize to keep ACT/DVE
    balanced (DVE's TT chain is the steady-state critical path; ACT's e2
    chain is second).
  * One output DMA per 4 rounds (24 total): 8KB-contiguous runs per
    partition; host un-permutes.
  * Softplus = Exp (+1) + batched Ln: exp and ln live in different ACT
    tables, so the Lns are batched into 3 sites (q0, q1-3, b1) to bound
    the 1.3us table-load thrash against the e2 Exps.
Startup: X arrives in 4 t-quarters per batch so score/softplus and the x2z
build pipeline against the DMA; first round issues ~5us in.
TimelineSim (cost-model) estimate: ~143us/core; CoreSim+HW rel err 4.3e-3.
"""

import numpy as np

import concourse.bass as bass
import concourse.tile as tile
from concourse import bacc, mybir
from concourse import bass_utils

F32 = mybir.dt.float32
BF16 = mybir.dt.bfloat16

# Problem shapes (hardcoded per contract)
S = 64          # sources
B = 16          # total batch
NCORES = 8
BL = B // NCORES  # batches per core = 2
F = 64          # features
T = 96          # time
D = 1024        # flattened target grid 32*32
DBLK = D // 128  # 8 d-blocks of 128
TP = T // 2     # 48 t-pairs
NQ = 4          # X arrives in 4 t-quarters per batch
QT = TP // NQ   # 12 t-pairs per quarter
TCH = 32        # t-chunk (stage tile holds 32 t values = 16 rounds)
NCH = T // TCH  # 3 chunks per batch
RPC = TCH // 2  # 16 rounds (t-pairs) per chunk

# MM_DT: dtype of matmul operands (e2 weights + moving x2z). bf16 halves
# weight-load time at ~0.4% relative error.
MM_DT = BF16
# OUT_DT: dtype of the staged/DMA'd output (host upcasts to f32).
OUT_DT = BF16
# Every Nth round bounces psum through SBUF via ACT-copy + GpSimd-normalize
# instead of the DVE drain (0 = never). Balances the ACT/DVE load.
BOUNCE_PERIOD = 7


def build_kernel():
    nc = bacc.Bacc("TRN2", target_bir_lowering=False, debug=False,
                   num_devices=NCORES)

    # xq[b, q, (th,s)=128, f, tq] = X[s, b, f, 2*(q*QT+tq)+th]
    x_t = nc.dram_tensor("xq", (BL, NQ, 128, F, QT), F32, kind="ExternalInput")
    # ndist_T[s, d] = -dist[d, s]  (host pre-transposed + negated)
    dist_t = nc.dram_tensor("ndist_T", (S, D), F32, kind="ExternalInput")
    w_t = nc.dram_tensor("w", (S, F), F32, kind="ExternalInput")
    bias_t = nc.dram_tensor("bias", (S, 1), F32, kind="ExternalInput")
    # Output in hardware-native layout (host un-permutes):
    # [b, ch, tlh, p, dblk, f, tlo] -> out[dblk*128+p, b, f, ch*32+tlh*8+tlo]
    out_t = nc.dram_tensor("out_hw", (BL, NCH, 4, 128, DBLK, F, 8),
                           OUT_DT, kind="ExternalOutput")

    def dram_ap(t, offset, ap):
        base = t.ap()
        return bass.AP(tensor=base.tensor, offset=offset, ap=ap)

    with tile.TileContext(nc) as tc:
        with (
            tc.tile_pool(name="statics", bufs=1) as statics,
            tc.tile_pool(name="ztp", bufs=2) as ztp,
            tc.tile_pool(name="e2p", bufs=6) as e2p,
            tc.tile_pool(name="stage", bufs=2) as stagep,
            tc.tile_pool(name="small", bufs=4) as small,
            tc.tile_pool(name="tmpp", bufs=4) as tmpp,
            tc.tile_pool(name="psum", bufs=3, space="PSUM") as psump,
            tc.tile_pool(name="denp", bufs=1, space="PSUM") as denp,
        ):
            # ---- static tiles -------------------------------------------
            # xsc[b][p=(th,s), q, f, tq]: f32 X for the score path
            xsc = [statics.tile([128, NQ, F, QT], F32, name=f"xsc{i}")
                   for i in range(BL)]
            # x2z[b][p=(th,s), tp, 128]: bf16 moving operand; cols z*64+f,
            # z==th rows hold X, z!=th rows are zero.
            x2z = [statics.tile([128, TP, 128], MM_DT, name=f"x2z{i}")
                   for i in range(BL)]
            # ones2[(th,s), z]: 1 where z==th -- moving operand of the tiny
            # denominator matmuls (den[d, t] = sum_s e2[(t,s), d])
            ones2 = statics.tile([128, 2], MM_DT)
            # ndist2[(th,s), d] = -dist(d,s), replicated across t-parity
            ndist2 = statics.tile([128, D], F32)
            w2 = statics.tile([128, F], F32)
            bias2 = statics.tile([128, 1], F32)
            sc = [statics.tile([128, TP], F32, name=f"sc{i}") for i in range(BL)]

            # ---- input DMAs (sync queue, in priority order) --------------
            def xq_dma(b, q):
                eng = nc.sync
                eng.dma_start(
                    out=xsc[b][:, q, :, :],
                    in_=dram_ap(x_t, (b * NQ + q) * 128 * F * QT,
                                [[F * QT, 128], [1, F * QT]]),
                )

            xq_dma(0, 0)
            for th in range(2):
                nc.sync.dma_start(
                    out=w2[th * S: (th + 1) * S, :],
                    in_=dram_ap(w_t, 0, [[F, S], [1, F]]),
                )
                nc.sync.dma_start(
                    out=bias2[th * S: (th + 1) * S, :],
                    in_=dram_ap(bias_t, 0, [[1, S], [0, 1]]),
                )
            for th in range(2):
                nc.sync.dma_start(
                    out=ndist2[th * S: (th + 1) * S, :],
                    in_=dram_ap(dist_t, 0, [[D, S], [1, D]]),
                )
            for q in range(1, NQ):
                xq_dma(0, q)
            for q in range(NQ):
                xq_dma(1, q)

            # softplus exp accumulators (Ln batched to avoid ACT
            # exp<->ln table thrash against the per-round e2 Exps)
            ez = [statics.tile([128, TP], F32, name=f"ez{i}")
                  for i in range(BL)]

            # ---- per-quarter score + x2z build --------------------------
            # x2z copies always on Pool; z/add on `veng`; reduce DVE-only.
            def build_quarter(b, q, veng):
                q0 = q * QT
                nc.gpsimd.tensor_copy(
                    x2z[b][0:S, q0:q0 + QT, 0:F],
                    xsc[b][0:S, q, :, :].rearrange("p f t -> p t f"),
                )
                nc.gpsimd.tensor_copy(
                    x2z[b][S:128, q0:q0 + QT, F:128],
                    xsc[b][S:128, q, :, :].rearrange("p f t -> p t f"),
                )
                # score: z = sum_f x*w
                zt = ztp.tile([128, QT, F], F32, tag=f"zt{b}")
                veng.tensor_tensor(
                    out=zt[:],
                    in0=xsc[b][:, q, :, :].rearrange("p f t -> p t f"),
                    in1=w2[:].unsqueeze(1).broadcast_to([128, QT, F]),
                    op=mybir.AluOpType.mult,
                )
                z = ztp.tile([128, QT], F32, tag=f"z{b}")
                # free-axis reduce is DVE-only (GpSimd can't)
                nc.vector.reduce_sum(out=z[:], in_=zt[:],
                                     axis=mybir.AxisListType.X)
                # ez = 1 + exp(z+bias)   (softplus part 1)
                nc.scalar.activation(
                    out=ez[b][:, q0:q0 + QT], in_=z[:],
                    func=mybir.ActivationFunctionType.Exp,
                    bias=bias2[:, 0:1], scale=1.0,
                )
                veng.tensor_scalar_add(
                    ez[b][:, q0:q0 + QT], ez[b][:, q0:q0 + QT], 1.0)

            def score_ln(b, q0, q1):
                # softplus part 2: sc = ln(ez), batched over quarters
                nc.scalar.activation(
                    out=sc[b][:, q0 * QT:q1 * QT], in_=ez[b][:, q0 * QT:q1 * QT],
                    func=mybir.ActivationFunctionType.Ln,
                )

            # b=0 zero quadrants on idle DVE at t=0 (big memsets, but DVE
            # has nothing else until the first xq quarter lands)
            nc.vector.memset(x2z[0][S:128, :, 0:F], 0.0)
            nc.vector.memset(x2z[0][0:S, :, F:128], 0.0)
            nc.vector.memset(ones2[0:S, 0:1], 1.0)
            nc.vector.memset(ones2[0:S, 1:2], 0.0)
            nc.vector.memset(ones2[S:128, 0:1], 0.0)
            nc.vector.memset(ones2[S:128, 1:2], 1.0)
            # b=1 zero quadrant A rides Pool's idle t=0 window
            nc.gpsimd.memset(x2z[1][S:128, :, 0:F], 0.0)
            build_quarter(0, 0, nc.vector)
            score_ln(0, 0, 1)
            # rest of b=1 x2z init between the b=0 quarter builds (Pool is
            # gated on the xq DMAs anyway)
            nc.gpsimd.memset(x2z[1][0:S, :, F:128], 0.0)
            for q in range(1, NQ):
                build_quarter(0, q, nc.vector)
            score_ln(0, 1, NQ)

            # ---- rounds --------------------------------------------------
            # Drains are deferred one round: round r emits recip(r) then
            # drain(r-1), so drain never waits on the recip->TT semaphore
            # (rc(r-1)'s update fired long ago).  Output DMAs shift one
            # round later to stay behind their last drain.
            # den[p, slot, dblk, t]: static 1-bank psum tile, 4 slots
            # rotating by round so the tiny den matmuls never wait on the
            # pair-reciprocal's reads (WAR tracked per-AP)
            den = denp.tile([128, 4, DBLK, 2], F32)
            stage = None
            pend_bounce = None  # (num_ap, rc_b, out_ap) for ACT+Pool path
            pend_drain = None   # (num_ap, rc_b, out_ap) for DVE path
            hp = DBLK * F * 8   # 4096 out elements per partition per tlh

            def emit_dma(st, b, ch, tlh):
                nc.sync.dma_start(
                    out=dram_ap(
                        out_t,
                        ((b * NCH + ch) * 4 + tlh) * 128 * hp,
                        [[hp, 128], [1, hp]],
                    ),
                    in_=st[:, tlh, :, :, :],
                )

            def emit_bounce(rec):
                p_num, p_rcb, p_out = rec
                tmp = tmpp.tile([128, DBLK, 2, F], F32, tag="bnc")
                nc.scalar.activation(
                    out=tmp[:], in_=p_num,
                    func=mybir.ActivationFunctionType.Copy,
                )
                nc.gpsimd.tensor_tensor(
                    out=p_out, in0=tmp[:], in1=p_rcb,
                    op=mybir.AluOpType.mult,
                )

            pend_drains = []  # (num_ap, out_ap, slot) awaiting pair recipB
            rcB = None

            for r in range(BL * TP):
                b, tp = divmod(r, TP)
                ch, rr = divmod(tp, RPC)
                tlh, r4 = divmod(rr, 4)
                tlo = 2 * r4

                if rr == 0:
                    stage_prev, stage = stage, stagep.tile(
                        [128, 4, DBLK, F, 8], OUT_DT)

                # e2[(th,s), d] = exp(score * -dist)
                e2 = e2p.tile([128, D], MM_DT)
                nc.scalar.activation(
                    out=e2[:], in_=ndist2[:],
                    func=mybir.ActivationFunctionType.Exp,
                    scale=sc[b][:, tp: tp + 1],
                )

                # pm[d%128=128p, dblk, 128]: cols t*64+f, exactly 2 PSUM
                # banks -> 3 pool bufs, so matmuls run a full round ahead.
                pm = psump.tile([128, DBLK, 128], F32, tag="pm")
                for dblk in range(DBLK):
                    nc.tensor.matmul(
                        out=pm[:, dblk, :],
                        lhsT=e2[:, dblk * 128: (dblk + 1) * 128],
                        rhs=x2z[b][:, tp, :],
                        start=True, stop=True,
                    )
                    # denominator: same stationary, ones moving
                    nc.tensor.matmul(
                        out=den[:, r % 4, dblk, :],
                        lhsT=e2[:, dblk * 128: (dblk + 1) * 128],
                        rhs=ones2[:],
                        start=True, stop=True,
                    )

                # one reciprocal per round PAIR (both den slots at once):
                # only one recip->drain semaphore wait per two drains
                if r % 2 == 1:
                    rcB = small.tile([128, 2, DBLK, 2], F32, tag="rc")
                    s0 = (r - 1) % 4
                    nc.vector.reciprocal(out=rcB[:], in_=den[:, s0:s0 + 2])

                # deferred bounce (ACT copy + Pool normalize): its pair's
                # rcB is emitted by now
                if pend_bounce is not None:
                    emit_bounce(pend_bounce)
                    pend_bounce = None

                # this round's drain record: stage[tlh, dblk, f, tlo+t]
                num_ap = pm[:].rearrange("p d (t x) -> p d t x", t=2)
                out_ap = stage[:, tlh, :, :, tlo: tlo + 2].rearrange(
                    "p d x t -> p d t x")
                pend_drains.append((num_ap, out_ap, r % 2,
                                    BOUNCE_PERIOD
                                    and r % BOUNCE_PERIOD == BOUNCE_PERIOD - 1))
                if r % 2 == 1:
                    for p_num, p_out, slot, bounce in pend_drains:
                        p_rcb = rcB[:, slot].unsqueeze(3).broadcast_to(
                            [128, DBLK, 2, F])
                        if bounce:
                            pend_bounce = (p_num, p_rcb, p_out)
                        else:
                            nc.vector.tensor_tensor(
                                out=p_out, in0=p_num, in1=p_rcb,
                                op=mybir.AluOpType.mult,
                            )
                    pend_drains = []

                # ---- output DMA, one round after its tlh's last drain ---
                # (flush any pending bounce first: the DMA may read the
                # stage slot the bounce writes)
                if rr in (4, 8, 12):
                    if pend_bounce is not None:
                        emit_bounce(pend_bounce)
                        pend_bounce = None
                    emit_dma(stage, b, ch, tlh - 1)
                elif rr == 0 and r > 0:
                    if pend_bounce is not None:
                        emit_bounce(pend_bounce)
                        pend_bounce = None
                    pb, pch = divmod((r - 1) // RPC, NCH)
                    emit_dma(stage_prev, pb, pch, 3)

                # b=1 prep on Pool, spread over early-round gaps (kept
                # clear of bounce rounds so bounces drain promptly)
                if r in (12, 17, 24, 31):
                    q = (12, 17, 24, 31).index(r)
                    build_quarter(1, q, nc.gpsimd)
                elif r == 33:
                    score_ln(1, 0, NQ)

            # tail: flush the last bounce and the final chunk DMA
            if pend_bounce is not None:
                emit_bounce(pend_bounce)
            emit_dma(stage, BL - 1, NCH - 1, 3)

    nc.compile()
    return nc


_NC_CACHE = None


def _get_nc():
    global _NC_CACHE
    if _NC_CACHE is None:
        _NC_CACHE = build_kernel()
    return _NC_CACHE


def make_inputs(X, dist, attention_weight, attention_bias):
    """Host-side marshaling: full inputs -> per-core input maps."""
    X = np.asarray(X, dtype=np.float32)                                # (S,B,F,T)
    dist_np = np.asarray(dist, dtype=np.float32).reshape(-1, S)        # (D,S)
    ndist_T = np.ascontiguousarray(-dist_np.T)                         # (S,D)
    w_np = np.ascontiguousarray(np.asarray(attention_weight, np.float32))
    bias_np = np.ascontiguousarray(
        np.asarray(attention_bias, np.float32).reshape(S, 1))
    # xq[b, q, (th,s), f, tq] = X[s, b, f, 2*(q*QT+tq)+th]
    # X -> (S, B, F, NQ, QT, 2) -> (B, NQ, 2, S, F, QT)
    xq_full = np.ascontiguousarray(
        X.reshape(S, B, F, NQ, QT, 2).transpose(1, 3, 5, 0, 2, 4))
    in_maps = []
    for c in range(NCORES):
        in_maps.append({
            "xq": np.ascontiguousarray(
                xq_full[c * BL: (c + 1) * BL]).reshape(BL, NQ, 128, F, QT),
            "ndist_T": ndist_T,
            "w": w_np,
            "bias": bias_np,
        })
    return in_maps


def unpack_out(hw):
    """out_hw (BL,NCH,4,128,DBLK,F,8) -> (D, BL, F, T) f32."""
    # [b, ch, tlh, p, dblk, f, tlo] -> [dblk, p, b, f, ch, tlh, tlo]
    return (np.asarray(hw).astype(np.float32)
            .transpose(4, 3, 0, 5, 1, 2, 6)
            .reshape(D, BL, F, T))


def kernel(X, dist, attention_weight, attention_bias):
    in_maps = make_inputs(X, dist, attention_weight, attention_bias)
    nc = _get_nc()
    res = bass_utils.run_bass_kernel_spmd(nc, in_maps, core_ids=list(range(NCORES)))
    out = np.empty((D, B, F, T), dtype=np.float32)
    for c in range(NCORES):
        out[:, c * BL: (c + 1) * BL] = unpack_out(res.results[c]["out_hw"])
    return out.reshape(32, 32, B, F, T)
